# revision 32
# baseline (speedup 1.0000x reference)
"""Tensor-parallel GQA multi-head-attention kernel for 8 trn2 NeuronCores.

Problem: B=2, T=2048, D=2048, H=16 q-heads, KV=4 kv-heads, HD=128,
causal attention with interleaved RoPE, y = attn_out @ Wo.

Sharding (tensor-parallel over heads, per the hint):
  core c = b*4 + g   (b = batch index, g = kv-head / q-head-group index)
  Each core computes q-heads 4g..4g+3 and kv-head g for batch b, plus the
  partial output  y_partial = attn_heads @ Wo[rows of those heads]  (row-
  parallel Wo).  The host sums the partials per batch (the unshard of the
  row-parallel all-reduce) and stacks the 2 batches.

v1 performance structure (vs the v0 baseline at ~320us):
  - inputs are host-packed chunk-contiguous so every load is ONE large DMA
    (128 partitions x 4-16KB contiguous), split across BOTH HWDGE queues
    (sync + scalar) in priority order -> input phase is bandwidth-bound,
    not descriptor-latency-bound.
  - ~9us of warmup matmuls on a zeroed tile overlap the input DMA and keep
    the PE HAM clock-gate at K=8/8 (2.4 GHz) from the start.
  - emission is slot-pipelined: attention blocks of chunk c are interleaved
    with projection matmuls of chunk c+1 and Wo matmuls of chunk c-1
    ("filler" MMs), and each block's PV/sums matmuls trail its S matmul by
    one block, so the PE never waits on the Scalar-engine exp
    ((N+352)/1.2ns latency) and the ACT load is spread over the whole slot.
  - Wo is computed in two head-pair passes writing separate bf16 outputs
    (y01 = heads 0,1; y23 = heads 2,3); pass 1 of the last chunk is issued
    mid-slot, so the end-of-kernel tail is only ~32 matmuls + stores.
    The host sums 4 bf16 partials per (batch, head-pair) in fp32.
"""

import math
import sys
from collections import defaultdict

import numpy as np

for _p in ("/opt/trn_rl_repo", "/root/.axon_site",
           "/root/.axon_site/_ro/trn_rl_repo",
           "/root/.axon_site/_ro/pypackages"):
    if _p not in sys.path:
        sys.path.append(_p)

B, T, D = 2, 2048, 2048
H, KV, HD = 16, 4, 128
ROPE_BASE = 10000.0
N_CORES = 8
HEADS_PER_CORE = 4
DQ = HEADS_PER_CORE * HD  # 512 q-dims per core
NK = D // 128             # contraction chunks for projections
SCALE = 1.0 / math.sqrt(HD)
MASK_VAL = -30000.0
N_WARM = 24

_CACHE = {}


class _Thunks:
    """A filler generator plus its remaining-yield count."""

    def __init__(self, gen, n):
        self.gen = gen
        self.n = n


def _build_nc(t_len=T):
    """Build the single-core SPMD Bass/Tile program (cached)."""
    import concourse.bass as bass
    import concourse.mybir as mybir
    import concourse.tile as tile
    from concourse import bacc

    f32 = mybir.dt.float32
    bf16 = mybir.dt.bfloat16
    f8 = mybir.dt.float8e4
    DR = mybir.MatmulPerfMode.DoubleRow
    ts = bass.ts

    NT = t_len // 128        # number of 128-row T tiles
    NCQ = t_len // 512       # number of 512-wide q chunks

    nc = bacc.Bacc("TRN2", target_bir_lowering=False, debug=False,
                   num_devices=N_CORES)

    xt_d = nc.dram_tensor("xt", [128, NCQ, NK, 512], bf16,
                          kind="ExternalInput").ap()
    wq_d = nc.dram_tensor("wq", [128, NK, DQ], bf16, kind="ExternalInput").ap()
    wk_d = nc.dram_tensor("wk", [128, NK, HD], bf16, kind="ExternalInput").ap()
    wv_d = nc.dram_tensor("wv", [128, NK, HD], bf16, kind="ExternalInput").ap()
    wo_d = nc.dram_tensor("wo", [128, HEADS_PER_CORE, D], bf16,
                          kind="ExternalInput").ap()
    cos_d = nc.dram_tensor("cosd", [128, t_len], bf16, kind="ExternalInput").ap()
    ssig_d = nc.dram_tensor("ssig", [128, t_len], bf16, kind="ExternalInput").ap()
    mask_d = nc.dram_tensor("mask", [128, 128], bf16, kind="ExternalInput").ap()
    perm_d = nc.dram_tensor("perm", [128, 128], bf16, kind="ExternalInput").ap()
    ident_d = nc.dram_tensor("ident", [128, 128], bf16, kind="ExternalInput").ap()
    y01_d = nc.dram_tensor("y01", [t_len, D], bf16, kind="ExternalOutput").ap()
    y23_d = nc.dram_tensor("y23", [t_len, D], bf16, kind="ExternalOutput").ap()

    Exp = mybir.ActivationFunctionType.Exp

    with tile.TileContext(nc) as tc:
        with (
            tc.tile_pool(name="const", bufs=1) as const,
            tc.tile_pool(name="qkv", bufs=1) as qkv,
            tc.tile_pool(name="attn", bufs=2) as attn_pool,
            tc.tile_pool(name="p", bufs=5) as p_pool,
            tc.tile_pool(name="p2", bufs=3) as p2_pool,
            tc.tile_pool(name="rope", bufs=2) as rope_pool,
            tc.tile_pool(name="recip", bufs=2) as recip_pool,
            tc.tile_pool(name="y", bufs=8) as y_pool,
            tc.tile_pool(name="psum", bufs=1, space="PSUM") as psum,
        ):
            # ---- warmup source (zeros) + PE warmup matmuls: keep the HAM
            # clock-gate busy/warm while the real inputs stream in ----
            wz = const.tile([128, 512], bf16, tag="wz")
            nc.gpsimd.memset(wz[:], 0.0)
            ones_sb = const.tile([128, 128], bf16, tag="ones")
            nc.gpsimd.memset(ones_sb[:], 1.0)
            ones2_sb = const.tile([128, 2, 128], f8, tag="ones2")
            nc.gpsimd.memset(ones2_sb[:], 1.0)
            for _ in range(N_WARM):
                wp = psum.tile([128, 512], f32, tag="mm", bufs=2, name="wp")
                nc.tensor.matmul(wp[:], wz[:, 0:128], wz[:],
                                 start=True, stop=True)

            # ---- input loads: one large DMA per tensor/chunk, split across
            # the two HWDGE queues (sync + scalar) in priority order ----
            xt_sb = const.tile([128, NCQ, NK, 512], bf16, tag="xt")
            wq_sb = const.tile([128, NK, DQ], bf16, tag="wq")
            wk_sb = const.tile([128, NK, HD], bf16, tag="wk")
            wv_sb = const.tile([128, NK, HD], bf16, tag="wv")
            wo_sb = const.tile([128, HEADS_PER_CORE, D], bf16, tag="wo")
            cos_sb = const.tile([128, t_len], bf16, tag="cos")
            ssig_sb = const.tile([128, t_len], bf16, tag="ssig")
            mask_sb = const.tile([128, 128], bf16, tag="mask")
            perm_sb = const.tile([128, 128], bf16, tag="perm")
            ident_sb = const.tile([128, 128], bf16, tag="ident")

            nc.sync.dma_start(xt_sb[:, 0, 0:NK // 2], xt_d[:, 0, 0:NK // 2])
            nc.sync.dma_start(wq_sb[:], wq_d[:])
            nc.scalar.dma_start(wk_sb[:], wk_d[:])
            nc.scalar.dma_start(xt_sb[:, 0, NK // 2:], xt_d[:, 0, NK // 2:])
            nc.scalar.dma_start(wv_sb[:], wv_d[:])
            nc.scalar.dma_start(cos_sb[:], cos_d[:])
            nc.scalar.dma_start(ssig_sb[:], ssig_d[:])
            nc.scalar.dma_start(mask_sb[:], mask_d[:])
            nc.scalar.dma_start(perm_sb[:], perm_d[:])
            nc.scalar.dma_start(ident_sb[:], ident_d[:])
            nc.scalar.dma_start(xt_sb[:, 1], xt_d[:, 1])
            nc.scalar.dma_start(wo_sb[:], wo_d[:])
            for c in range(2, NCQ):
                nc.scalar.dma_start(xt_sb[:, c], xt_d[:, c])

            # persistent activations
            qT = qkv.tile([128, HEADS_PER_CORE, t_len], bf16, tag="qT")
            kT = qkv.tile([128, t_len], bf16, tag="kT")
            v_sb = qkv.tile([128, NT, HD], bf16, tag="v")
            # fp8 copies of the v tiles that appear as full (off-diagonal)
            # blocks, stored as DoubleRow pairs: v8[:, j//2, j%2, :] = v_j
            v8_sb = qkv.tile([128, (NT - 4) // 2, 2, HD], f8, tag="v8")

            def rope_begin(dst_ap, psum_tile, c):
                """Stage 1 of RoPE: drain the projection psum to SBUF
                (bf16).  Stage 2 (rope_finish) is emitted a few matmuls
                later so the PE's half-swap matmul never head-of-line
                blocks on this DVE copy."""
                qf = rope_pool.tile([128, 512], bf16, tag="qf")
                nc.vector.tensor_copy(qf[:], psum_tile[:])
                return (dst_ap, qf, c)

            def rope_finish(st):
                dst_ap, qf, c = st
                cs = slice(c * 512, (c + 1) * 512)
                sw = psum.tile([128, 512], f32, tag="sw", bufs=1, name="sw")
                nc.tensor.matmul(sw[:], perm_sb[:], qf[:],
                                 start=True, stop=True)
                qc = rope_pool.tile([128, 512], bf16, tag="qc")
                nc.gpsimd.tensor_mul(qc[:], qf[:], cos_sb[:, cs])
                qs2 = rope_pool.tile([128, 512], bf16, tag="qs2")
                nc.vector.tensor_mul(qs2[:], sw[:], ssig_sb[:, cs])
                nc.gpsimd.tensor_add(dst_ap, qc[:], qs2[:])

            def gen_proj(c, v_first=False):
                """Projection matmuls for chunk c, one yield per matmul.
                v_first: V before Q (chunk 0: wq is still loading); otherwise
                q0/q1 right after k so their rope latency hides under the
                remaining groups."""
                cs = slice(c * 512, (c + 1) * 512)
                pend_rope = []
                delay = [0]

                def tick():
                    # countdown toward finishing a staged rope
                    if pend_rope:
                        delay[0] -= 1
                        if delay[0] <= 0:
                            rope_finish(pend_rope.pop(0))

                def stage(st):
                    while pend_rope:
                        rope_finish(pend_rope.pop(0))
                    pend_rope.append(st)
                    delay[0] = 9

                kp = psum.tile([128, 512], f32, tag="acc", bufs=2, name="kp")
                for k in range(NK):
                    nc.tensor.matmul(kp[:], wk_sb[:, k, :], xt_sb[:, c, k, :],
                                     start=(k == 0), stop=(k == NK - 1))
                    if k < NK - 1:
                        tick()
                        yield
                stage(rope_begin(kT[:, cs], kp, c))
                yield
                order = (["v0", "v1", "v2", "v3", "q0", "q1", "q2", "q3"]
                         if v_first else
                         ["q0", "q1", "v0", "v1", "v2", "v3", "q2", "q3"])
                for item in order:
                    if item[0] == "q":
                        h = int(item[1])
                        qp = psum.tile([128, 512], f32, tag="acc", bufs=2,
                                       name="qp")
                        for k in range(NK):
                            nc.tensor.matmul(qp[:], wq_sb[:, k, ts(h, 128)],
                                             xt_sb[:, c, k, :],
                                             start=(k == 0),
                                             stop=(k == NK - 1))
                            if k < NK - 1:
                                tick()
                                yield
                        stage(rope_begin(qT[:, h, cs], qp, c))
                        yield
                    else:
                        tl = int(item[1])
                        vp = psum.tile([128, 512], f32, tag="acc", bufs=2,
                                       name="vp")
                        for k in range(NK):
                            nc.tensor.matmul(vp[:, 0:128],
                                             xt_sb[:, c, k, ts(tl, 128)],
                                             wv_sb[:, k, :],
                                             start=(k == 0),
                                             stop=(k == NK - 1))
                            if k < NK - 1:
                                tick()
                                yield
                        tt = 4 * c + tl
                        nc.vector.tensor_copy(v_sb[:, tt, :], vp[:, 0:128])
                        if tt < NT - 4:
                            nc.vector.tensor_copy(v8_sb[:, tt // 2, tt % 2, :],
                                                  vp[:, 0:128])
                        yield
                while pend_rope:
                    rope_finish(pend_rope.pop(0))

            PROJ_YIELDS = NK * (1 + HEADS_PER_CORE + 4)

            def gen_wo(c, attn_t, heads, yd, tag="acc", bufs=2, alt=False):
                """Output-projection matmuls for chunk c over `heads`,
                accumulated into output tensor `yd`.  tag=None cycles the
                psum rings (endgame: all rings are free)."""
                cyc = ("mm", "acc", "out")
                for gi, (nn, tq) in enumerate(
                        (n, t) for n in range(4) for t in range(4)):
                    tg = tag or cyc[gi % 3]
                    yp = psum.tile([128, 512], f32, tag=tg, bufs=2,
                                   name="yp")
                    for hi, h in enumerate(heads):
                        nc.tensor.matmul(yp[:], attn_t[:, h, ts(tq, 128)],
                                         wo_sb[:, h, ts(nn, 512)],
                                         start=(hi == 0),
                                         stop=(hi == len(heads) - 1))
                        if hi < len(heads) - 1:
                            yield
                    row0 = (4 * c + tq) * 128
                    ysb = y_pool.tile([128, 512], bf16, tag="y")
                    if alt:
                        # split the psum->sbuf cast across DVE+ACT; endgame
                        # (alt=1) also alternates store queues
                        nc.vector.tensor_copy(ysb[:, 0:256], yp[:, 0:256])
                        nc.scalar.copy(ysb[:, 256:512], yp[:, 256:512])
                        eng = nc.scalar if (alt == 1 and gi % 2) else nc.sync
                        eng.dma_start(yd[row0:row0 + 128, ts(nn, 512)],
                                      ysb[:])
                    else:
                        nc.vector.tensor_copy(ysb[:], yp[:])
                        nc.sync.dma_start(yd[row0:row0 + 128, ts(nn, 512)],
                                          ysb[:])
                    yield



            def emit_attn_slot(c, attn_t, head_fillers):
                """Attention for chunk c (4 heads), with PV/sums trailing one
                block behind S/exp and filler matmuls pumped in between."""
                nj = 4 * c + 4
                nb = nj * HEADS_PER_CORE
                avail = []
                state = {"rem": 0, "rr": 0}

                def pump(kmax):
                    done = 0
                    while done < kmax and avail:
                        idx = state["rr"] % len(avail)
                        it = avail[idx]
                        try:
                            next(it.gen)
                            it.n -= 1
                            state["rem"] -= 1
                            done += 1
                            state["rr"] += 1
                        except StopIteration:
                            avail.pop(idx)
                    return done

                def emit_pv(e):
                    (h, payload, out_ps, sums_ps, first, last) = e
                    if payload[0] == "pair":
                        _, a, p2 = payload
                        nc.tensor.matmul(out_ps[:], v8_sb[:, a], p2[:],
                                         start=first, stop=False,
                                         perf_mode=DR)
                        nc.tensor.matmul(sums_ps[:], ones2_sb[:], p2[:],
                                         start=first, stop=False,
                                         perf_mode=DR)
                    else:
                        _, j, pp, lo = payload
                        nc.tensor.matmul(out_ps[:, lo:], v_sb[:, j, :],
                                         pp[:, lo:], start=first, stop=last)
                        nc.tensor.matmul(sums_ps[:, lo:], ones_sb[:],
                                         pp[:, lo:], start=first, stop=last)
                    if last:
                        rc = recip_pool.tile([128, 512], f32, tag="rc")
                        nc.vector.reciprocal_approx_fast(out=rc[:],
                                                         in_=sums_ps[:])
                        nc.vector.tensor_mul(attn_t[:, h, :], out_ps[:], rc[:])

                pend = []
                bi = 0
                nu_head = 2 * c + 4
                nb = nu_head * HEADS_PER_CORE
                for h in range(HEADS_PER_CORE):
                    for f in head_fillers.get(h, ()):
                        avail.append(f)
                        state["rem"] += f.n
                    if h == 0:
                        pump(12)
                    out_ps = psum.tile([128, 512], f32, tag="out", bufs=2,
                                       name="out")
                    sums_ps = psum.tile([128, 512], f32, tag="sums", bufs=1,
                                        name="sums")
                    units = ([("pair", a) for a in range(2 * c)]
                             + [("diag", j) for j in range(4 * c, nj)])
                    for ui, unit in enumerate(units):
                        if unit[0] == "pair":
                            a = unit[1]
                            p2 = p2_pool.tile([128, 2, 512], f8, tag="p2")
                            for i in (0, 1):
                                jj = 2 * a + i
                                s_ps = psum.tile([128, 512], f32, tag="mm",
                                                 bufs=2, name="s")
                                nc.tensor.matmul(s_ps[:], kT[:, ts(jj, 128)],
                                                 qT[:, h,
                                                    c * 512:(c + 1) * 512],
                                                 start=True, stop=True)
                                nc.scalar.activation(p2[:, i, :], s_ps[:],
                                                     Exp, bias=0.0,
                                                     scale=SCALE)
                            payload = ("pair", a, p2)
                        else:
                            j = unit[1]
                            o = j - 4 * c
                            lo = o * 128
                            qs0 = c * 512 + lo
                            s_ps = psum.tile([128, 512], f32, tag="mm",
                                             bufs=2, name="s")
                            nc.tensor.matmul(s_ps[:, lo:], kT[:, ts(j, 128)],
                                             qT[:, h, qs0:(c + 1) * 512],
                                             start=True, stop=False)
                            # causal mask for the diagonal block, as a cheap
                            # N=128 accumulating matmul: I.T @ mask
                            nc.tensor.matmul(s_ps[:, lo:lo + 128],
                                             ident_sb[:], mask_sb[:],
                                             start=False, stop=True)
                            pp = p_pool.tile([128, 512], bf16, tag="p",
                                             name="pp")
                            nc.scalar.activation(pp[:, lo:], s_ps[:, lo:],
                                                 Exp, bias=0.0, scale=SCALE)
                            payload = ("diag", j, pp, lo)
                        blocks_left = nb - bi
                        k = min(3, max(1, state["rem"] // blocks_left))
                        pump(k)
                        if len(pend) >= 2:
                            emit_pv(pend.pop(0))
                        pend.append((h, payload, out_ps, sums_ps, ui == 0,
                                     ui == len(units) - 1))
                        bi += 1
                        if ui == len(units) - 1:
                            while pend:
                                emit_pv(pend.pop(0))
                                if pend:
                                    pump(2)
                                pump(10 ** 9)

            # ---- emission: proj(0) dense, then pipelined slots ----
            for _ in gen_proj(0, v_first=True):
                pass

            attn_ts = {}
            for c in range(NCQ):
                attn_ts[c] = attn_pool.tile([128, HEADS_PER_CORE, 512], bf16,
                                            tag="attnT", name=f"attnT{c}")

            for c in range(NCQ):
                hf = defaultdict(list)
                if c + 1 < NCQ:
                    hf[0].append(_Thunks(gen_proj(c + 1), PROJ_YIELDS))
                if c >= 1:
                    w_alt = 0
                    hf[0].append(_Thunks(gen_wo(c - 1, attn_ts[c - 1],
                                                (0, 1, 2, 3), y01_d,
                                                alt=w_alt), 64))
                if c == NCQ - 1:
                    # last chunk: heads 0,1 finish mid-slot, so their Wo pass
                    # can fill the tail of the slot
                    hf[2].append(_Thunks(gen_wo(c, attn_ts[c], (0, 1),
                                                y01_d), 32))
                emit_attn_slot(c, attn_ts[c], hf)

            # final tail: heads 2,3 of the last chunk (mm psum ring is free
            # by now; alternate the psum->sbuf copies across DVE/ACT)
            for _ in gen_wo(NCQ - 1, attn_ts[NCQ - 1], (2, 3), y23_d,
                            tag=None, alt=True):
                pass

    nc.finalize()
    return nc


def _prep_inputs(x, Wq, Wk, Wv, Wo, t_len=T):
    """Host-side shard + layout prep -> per-core input maps."""
    import ml_dtypes
    bf16 = ml_dtypes.bfloat16

    NCQ = t_len // 512

    x = np.asarray(x, np.float32)
    Wq = np.asarray(Wq, np.float32)
    Wk = np.asarray(Wk, np.float32)
    Wv = np.asarray(Wv, np.float32)
    Wo = np.asarray(Wo, np.float32)

    # RoPE de-interleave permutation within one head: [evens | odds]
    perm = np.concatenate([np.arange(0, HD, 2), np.arange(1, HD, 2)])

    # rope tables (match reference: freqs = t * base**(-2j/HD))
    inv = 1.0 / (ROPE_BASE ** (np.arange(0, HD, 2, dtype=np.float32) / HD))
    tpos = np.arange(t_len, dtype=np.float32)
    f = inv[:, None] * tpos[None, :]                       # [64, T]
    cos_dup = np.concatenate([np.cos(f), np.cos(f)], 0).astype(bf16)
    ssig = np.concatenate([-np.sin(f), np.sin(f)], 0).astype(bf16)

    # strict-lower-triangular causal mask template for the diagonal
    # [tk-tile, tq-tile] block (tk > tq within the 128x128 block)
    r = np.arange(128)[:, None]
    col = np.arange(128)[None, :]
    mask_t = np.where(r > col, MASK_VAL, 0.0).astype(bf16)
    perm_m = np.zeros((128, 128), np.float32)
    perm_m[np.arange(128), (np.arange(128) + 64) % 128] = 1.0
    perm_m = perm_m.astype(bf16)
    ident_m = np.eye(128, dtype=np.float32).astype(bf16)

    # chunk-contiguous xT packing: [128, NCQ, NK, 512]
    xt_b = []
    for b in range(B):
        a = x[b, :t_len].T.reshape(NK, 128, NCQ, 512).transpose(1, 2, 0, 3)
        xt_b.append(np.ascontiguousarray(a).astype(bf16))

    in_maps = []
    for b in range(B):
        for g in range(KV):
            wq_g = Wq[:, g * DQ:(g + 1) * DQ].reshape(D, HEADS_PER_CORE, HD)
            wq_g = wq_g[:, :, perm].reshape(NK, 128, DQ).transpose(1, 0, 2)
            wk_g = Wk[:, g * HD:(g + 1) * HD][:, perm]
            wk_g = wk_g.reshape(NK, 128, HD).transpose(1, 0, 2)
            wv_g = Wv[:, g * HD:(g + 1) * HD]
            wv_g = wv_g.reshape(NK, 128, HD).transpose(1, 0, 2)
            wo_g = Wo[g * DQ:(g + 1) * DQ, :]
            wo_g = wo_g.reshape(HEADS_PER_CORE, 128, D).transpose(1, 0, 2)
            in_maps.append({
                "xt": xt_b[b],
                "wq": np.ascontiguousarray(wq_g).astype(bf16),
                "wk": np.ascontiguousarray(wk_g).astype(bf16),
                "wv": np.ascontiguousarray(wv_g).astype(bf16),
                "wo": np.ascontiguousarray(wo_g).astype(bf16),
                "cosd": cos_dup, "ssig": ssig, "mask": mask_t, "perm": perm_m, "ident": ident_m,
            })
    return in_maps


def run(inputs, trace=False, t_len=T):
    """Run the sharded kernel; returns (y_full, BassKernelResults)."""
    from concourse.bass_utils import run_bass_kernel_spmd

    key = ("nc", t_len)
    if key not in _CACHE:
        _CACHE[key] = _build_nc(t_len)
    nc = _CACHE[key]

    in_maps = _prep_inputs(inputs["x"], inputs["Wq"], inputs["Wk"],
                           inputs["Wv"], inputs["Wo"], t_len)
    res = run_bass_kernel_spmd(nc, in_maps, list(range(N_CORES)), trace=trace)

    y = np.empty((B, t_len, D), np.float32)
    for b in range(B):
        acc = np.zeros((t_len, D), np.float32)
        lastc = t_len - 512
        for g in range(KV):
            r = res.results[b * KV + g]
            acc += np.asarray(r["y01"], np.float32)
            acc[lastc:] += np.asarray(r["y23"][lastc:], np.float32)
        y[b] = acc
    return y, res


def kernel(**inputs) -> np.ndarray:
    y, _ = run(inputs, trace=False)
    return y
